# revision 55
# baseline (speedup 1.0000x reference)
# Multi-head attention (B=4, S=2048, D=1024, H=16) on 8 NeuronCores.
#
# Sharding: batch x head-group. Core c handles batch b=c//2 and heads
# 8*(c%2) .. 8*(c%2)+7 (a 512-wide slice of the model dim). Each core
# computes QKV projections for its slice, causal attention for its 8
# heads, and a row-parallel partial of the output projection. The host
# sums the two partials per batch and adds bo.
#
# Precision/engine split:
#  - Q/K projections and the QK^T scores run in fp8 e4m3 with DoubleRow
#    perf mode (two contraction tiles per pass). x and Wq/Wk are scaled
#    by 16 on the host so their products sit in e4m3's happy range; the
#    combined 256x score scale is folded into the softmax exp scale.
#    Softmax is insensitive to the ~2-3% fp8 score noise (it perturbs
#    probabilities multiplicatively and averages out over the value
#    contraction).
#  - The value path (x/Wv/V/probs/context/Wo all bf16) keeps higher
#    precision: value-path error lands directly in the output.
#  - k's bias is dropped: it shifts every score for a given query
#    equally, which softmax removes. q's bias is folded into the
#    PSUM->SBUF fp8 cast (one tensor_scalar mult+add on DVE).
#  - The q/k fp8 SBUF layout packs each head's 64 contraction dims as
#    [32 partitions x 2 DoubleRow k-tiles], four heads per 128
#    partitions, via a host-side permutation of the Wq/Wk columns (the
#    same permutation on q and k leaves q.k unchanged), so scores need
#    no on-chip reshuffling.
#  - Softmax denominators come from 64 ones-columns carried in each
#    head's V block (the AV matmul accumulates them on PSUM partitions
#    64..127); 1/sum runs on the DVE (InstReciprocal), keeping the
#    scalar engine exclusively on the big exp.
#  - Output-projection PSUM->SBUF copies run on GPSIMD, which is
#    otherwise idle.
#
# Causality is exploited at three granularities: fully-masked k-blocks
# are skipped, partially-masked diagonal blocks trim the scores/AV
# matmuls to the valid column range, and the in-block triangle is
# zeroed with one [128,128] upper-tri mask multiply.
#
# Emission is a software-pipelined merge: per q-chunk, each head's
# score groups stream with their AV matmuls lagging a few groups
# behind, and projection/output-projection work is interleaved as PE
# filler (gated on estimated DMA arrival) so the PE stays busy while
# the scalar engine runs the exps. Input DMAs issue on one queue in
# strict deadline order (the HWDGE serializes transfers across queues).
# Cost-model makespan ~204 us/core vs 275 us for the fp32r baseline.

import sys

for _p in ("/opt/trn_rl_repo", "/root/.axon_site/_ro/trn_rl_repo"):
    if _p not in sys.path:
        sys.path.append(_p)

import ml_dtypes
import numpy as np

import concourse.bass as bass
import concourse.mybir as mybir
import concourse.tile as tile
from concourse.bass_utils import run_bass_kernel_spmd
from concourse.masks import make_upper_triangular

B, S, D, H = 4, 2048, 1024, 16
HD = D // H            # 64
N_CORES = 8
GH = 8                 # heads per core
C = GH * HD            # 512 local model dims per core
SCALE = HD ** -0.5
F32 = mybir.dt.float32
F32R = mybir.dt.float32r
BF16 = mybir.dt.bfloat16
FP8 = mybir.dt.float8e4
DR = mybir.MatmulPerfMode.DoubleRow
NP_FP8 = ml_dtypes.float8_e4m3

QK_SCALE = 16.0        # host-side scale on x and Wq/Wk before fp8 cast
import os as _os
WAIT_SCALE = float(_os.environ.get("KWAIT", "1.0"))

PE_NS = 1.0 / 2.4      # ns per matmul output row at full clock
ACT_NS = 1.0 / 1.2     # ns per activation element-row

T_CHUNK = 512          # t-tile for QKV projections
QC = 512               # q columns per attention chunk
KB = 128               # k rows per attention block
N_KB = S // KB         # 16
N_QC = S // QC         # 4
EXP_GROUP = 2          # k-blocks per batched exp (2 psum banks x 2 bufs)


def _split_multi_waits(nc):
    """walrus in this container accepts only one sync-wait per instruction.
    Hoist all but the last wait of any multi-wait instruction onto NoOps
    inserted just before it on the same engine (sequencers execute their
    queue in order, so chained single waits are equivalent)."""
    for f in nc.m.functions:
        for blk in f.blocks:
            new_insts = []
            for inst in blk.instructions:
                si = inst.sync_info
                if si is not None and si.on_wait and len(si.on_wait) > 1:
                    waits = list(si.on_wait)
                    for i, w in enumerate(waits[:-1]):
                        nop = mybir.InstNoOp(name=f"{inst.name}_sw{i}", ins=[], outs=[])
                        nop.engine = inst.engine
                        nop.sync_info = mybir.SyncInfo(on_wait=[w], on_update=[])
                        new_insts.append(nop)
                    si.on_wait = [waits[-1]]
                new_insts.append(inst)
            blk.instructions[:] = new_insts


def _emit_kernel(nc, reps=1):
    xt = nc.dram_tensor("xt", [D, S], BF16, kind="ExternalInput").ap()
    x8t = nc.dram_tensor("x8t", [D, S], FP8, kind="ExternalInput").ap()
    wq8 = nc.dram_tensor("wq8", [D, C], FP8, kind="ExternalInput").ap()
    wk8 = nc.dram_tensor("wk8", [D, C], FP8, kind="ExternalInput").ap()
    wvt = nc.dram_tensor("wvt", [D, C], BF16, kind="ExternalInput").ap()
    bqv = nc.dram_tensor("bqv", [C], F32, kind="ExternalInput").ap()
    bvb = nc.dram_tensor("bvb", [128, C], F32, kind="ExternalInput").ap()
    wot = nc.dram_tensor("wot", [C, D], BF16, kind="ExternalInput").ap()
    out = nc.dram_tensor("out", [S, D], F32, kind="ExternalOutput").ap()

    NFC = D // 128      # 8 f-chunks of the projection contraction
    NFP = NFC // 2      # 4 DoubleRow fc-pairs
    NCC = C // 128      # 4 c-chunks of the local model dim

    with tile.TileContext(nc) as tc:
        import contextlib

        ctx = contextlib.ExitStack()
        with ctx:
            consts = ctx.enter_context(tc.tile_pool(name="consts", bufs=1))
            wpool = ctx.enter_context(tc.tile_pool(name="wpool", bufs=1))
            qkv = ctx.enter_context(tc.tile_pool(name="qkv", bufs=1))
            xtp = ctx.enter_context(tc.tile_pool(name="xtp", bufs=2))
            expp = ctx.enter_context(tc.tile_pool(name="expp", bufs=12))
            ctxp = ctx.enter_context(tc.tile_pool(name="ctxp", bufs=1))
            smallp = ctx.enter_context(tc.tile_pool(name="smallp", bufs=1))
            outp = ctx.enter_context(tc.tile_pool(name="outp", bufs=3))

            ps_qkv = ctx.enter_context(
                tc.tile_pool(name="ps_qkv", bufs=2, space="PSUM")
            )
            ps_sc = ctx.enter_context(
                tc.tile_pool(name="ps_sc", bufs=2, space="PSUM")
            )
            ps_av = ctx.enter_context(
                tc.tile_pool(name="ps_av", bufs=2, space="PSUM")
            )

            # ---- constants -------------------------------------------------
            tri = consts.tile([128, 128], BF16)      # tri[p, c] = 1.0 iff p <= c
            make_upper_triangular(nc, tri[:, :], val=1.0, diag=True)

            # ---- inputs: one queue (scalar), strict deadline order --------
            # The HWDGE serializes all queues' transfers, so ordering is only
            # controllable within one queue. First-exp path (x8 head, wq8,
            # wk8, bq) first; then the value path (xt0, wv) and the rest by
            # next-use time. Mid-stream xt loads go on sync (the scalar
            # queue carries exps by then and must not block on DMA).
            x8_sb = wpool.tile([128, NFC, S], FP8)   # full 16*x, fp8
            _x8_ap = x8t.rearrange("(fc p) t -> p fc t", p=128)
            nc.scalar.dma_start(out=x8_sb[:, :, 0 : S // 4], in_=_x8_ap[:, :, 0 : S // 4])
            wq8_sb = wpool.tile([128, NFC, C], FP8)
            wk8_sb = wpool.tile([128, NFC, C], FP8)
            nc.scalar.dma_start(out=wq8_sb[:, :, :], in_=wq8.rearrange("(fc p) c -> p fc c", p=128))
            nc.scalar.dma_start(out=wk8_sb[:, :, :], in_=wk8.rearrange("(fc p) c -> p fc c", p=128))
            bq_sb = consts.tile([128, NCC], F32)    # 16*bq[perm][cc*128 + p] at [p, cc]
            nc.scalar.dma_start(out=bq_sb[:, :], in_=bqv.rearrange("(cc p) -> p cc", p=128))

            _xt_pref = {}

            def _xt_prefetch(tci, queue):
                xt_c = xtp.tile([128, NFC, T_CHUNK], BF16, name="xt_c")
                queue.dma_start(
                    out=xt_c[:, :, :],
                    in_=xt.rearrange("(fc p) t -> p fc t", p=128)[
                        :, :, tci * T_CHUNK : (tci + 1) * T_CHUNK
                    ],
                )
                _xt_pref[tci] = xt_c

            _xt_prefetch(0, nc.scalar)
            wv_sb = wpool.tile([128, NFC, C], BF16)
            nc.scalar.dma_start(out=wv_sb[:, :, :], in_=wvt.rearrange("(fc p) c -> p fc c", p=128))
            nc.scalar.dma_start(out=x8_sb[:, :, S // 4 : S // 2], in_=_x8_ap[:, :, S // 4 : S // 2])
            bv_bc = consts.tile([128, C], F32)      # bv broadcast across partitions
            nc.scalar.dma_start(out=bv_bc[:, :], in_=bvb)
            _xt_prefetch(1, nc.scalar)
            nc.scalar.dma_start(out=x8_sb[:, :, S // 2 : 3 * S // 4], in_=_x8_ap[:, :, S // 2 : 3 * S // 4])
            nc.scalar.dma_start(out=x8_sb[:, :, 3 * S // 4 : S], in_=_x8_ap[:, :, 3 * S // 4 : S])
            wo_sb = wpool.tile([128, NCC, D], BF16)
            nc.scalar.dma_start(out=wo_sb[:, :, :], in_=wot.rearrange("(cc p) d -> p cc d", p=128))

            # ---- persistent activations -----------------------------------
            # q/k fp8: [32*quad partitions, chunk-pair e, d-half j, t];
            # head h = 4e + quad, its 64 d-dims at partitions 32*quad..+32,
            # split into two DoubleRow k-tiles along j.
            qt8_sb = qkv.tile([128, 2, 2, S], FP8)
            kt8_sb = qkv.tile([128, 2, 2, S], FP8)
            # APs cannot start at partition 96, so quad-3 heads (h=3,7) get
            # their q/k relocated to partitions 0-63 of side tiles via
            # SBUF->SBUF DMA (the only engine that can shift partitions).
            qx_sb = qkv.tile([64, 2, S], FP8)   # [32*e + r, d-half, t]
            kx_sb = qkv.tile([64, 2, S], FP8)
            v_sb = qkv.tile([128, N_KB, GH, 2 * HD], BF16)  # v + 64 ones cols
            ctx_sb = ctxp.tile([128, NCC, S], BF16)  # ctxT: [c within chunk, cc, q]

            nc.gpsimd.memset(v_sb[:, :, :, HD : 2 * HD], 1.0)

            _phases = "123"
            _xt_tiles = {}

            def emit_xt_load(tci):
                if tci not in _xt_pref:
                    _xt_prefetch(tci, nc.sync)
                _xt_tiles[tci] = _xt_pref.pop(tci)
                return 0.0

            def emit_qk_group(tci, is_q, cc):
                """One 128-col psum group of the q (or k) fp8 DoubleRow
                projection + its PSUM->fp8 SBUF cast; quad-3 relocation DMA
                after the last chunk."""
                t0 = tci * T_CHUNK
                w_sb, y_sb = (wq8_sb, qt8_sb) if is_q else (wk8_sb, kt8_sb)
                e, j = cc // 2, cc % 2
                ps = ps_qkv.tile([128, T_CHUNK], F32, name="ps_qk", tag="ps_qkv")
                # moving free dim caps at 512 (2x256 under DoubleRow), so a
                # 512-t chunk takes two matmuls per fc-pair
                TH = 256
                for th in range(T_CHUNK // TH):
                    ts0 = t0 + th * TH
                    for fp in range(NFP):
                        nc.tensor.matmul(
                            ps[:, th * TH : (th + 1) * TH],
                            w_sb[:, 2 * fp : 2 * fp + 2, cc * 128 : (cc + 1) * 128],
                            x8_sb[:, 2 * fp : 2 * fp + 2, ts0 : ts0 + TH],
                            start=(fp == 0),
                            stop=(fp == NFP - 1),
                            perf_mode=DR,
                        )
                # psum holds 256*(proj); store 16*(proj+bias) as fp8
                if is_q:
                    nc.vector.tensor_scalar(
                        y_sb[:, e, j, t0 : t0 + T_CHUNK],
                        ps[:, :],
                        1.0 / QK_SCALE,
                        bq_sb[:, cc : cc + 1],
                        op0=mybir.AluOpType.mult,
                        op1=mybir.AluOpType.add,
                    )
                else:
                    nc.vector.tensor_scalar_mul(
                        y_sb[:, e, j, t0 : t0 + T_CHUNK], ps[:, :], 1.0 / QK_SCALE
                    )
                if j == 1:
                    # both d-halves of chunk-pair e are now cast: relocate
                    # its quad-3 head rows (base partition 96 is not
                    # AP-addressable) to the side tile
                    x_sb = qx_sb if is_q else kx_sb
                    nc.gpsimd.dma_start(
                        out=x_sb[32 * e : 32 * e + 32, :, t0 : t0 + T_CHUNK],
                        in_=y_sb[96:128, e, :, t0 : t0 + T_CHUNK],
                    )
                return NFP * T_CHUNK * 0.5 * PE_NS

            def emit_v_group(tci, tt):
                t0 = tci * T_CHUNK
                if tci not in _xt_tiles:
                    emit_xt_load(tci)
                xt_c = _xt_tiles[tci]
                kb = (t0 + tt * 128) // KB
                ps = ps_qkv.tile([128, C], F32, name="ps_v", tag="ps_qkv")
                for fc in range(NFC):
                    nc.tensor.matmul(
                        ps[:, :],
                        xt_c[:, fc, tt * 128 : (tt + 1) * 128],
                        wv_sb[:, fc, :],
                        start=(fc == 0),
                        stop=(fc == NFC - 1),
                    )
                nc.vector.tensor_add(
                    v_sb[:, kb, :, 0:HD],
                    ps.rearrange("p (h d) -> p h d", h=GH),
                    bv_bc.rearrange("p (h d) -> p h d", h=GH),
                )
                if tt == T_CHUNK // 128 - 1:
                    _xt_tiles.pop(tci)
                return NFC * C * PE_NS

            av_tiles = {}

            def eg_of(qi):
                return EXP_GROUP

            def sc_widths(qi, gi):
                """Matmul column-chunk widths for scores group gi at qi."""
                eg = eg_of(qi)
                nkb = 4 * qi + 4
                g_min = gi * eg - 4 * qi
                g_off0 = 128 * g_min if g_min > 0 else 0
                widths = []
                for kb in range(gi * eg, min((gi + 1) * eg, nkb)):
                    qq = g_off0
                    while qq < QC:
                        w = min(256, QC - qq)
                        widths.append(w)
                        qq += w
                return widths

            def emit_sc_group(h, qi, gi):
                """Scores + exp + tri-mask for k-group gi of head h. Returns
                the state emit_av_group needs, so AV can lag one group behind
                and the PE never stalls on the scalar engine's exp."""
                e, a = h // 4, h % 4
                if a < 3:
                    p0 = 32 * a
                    q_ap = lambda c0, c1: qt8_sb[p0 : p0 + 32, e, :, c0:c1]
                    k_ap = lambda c0, c1: kt8_sb[p0 : p0 + 32, e, :, c0:c1]
                else:
                    p0 = 32 * e
                    q_ap = lambda c0, c1: qx_sb[p0 : p0 + 32, :, c0:c1]
                    k_ap = lambda c0, c1: kx_sb[p0 : p0 + 32, :, c0:c1]
                q0 = qi * QC
                eg = eg_of(qi)
                nkb = 4 * qi + 4
                kb_lo = gi * eg
                kb_hi = min(kb_lo + eg, nkb)
                gw = kb_hi - kb_lo
                if gi == 0:
                    av_tiles[h] = ps_av.tile([128, QC], F32, name="av_ps")
                sc_ps = ps_sc.tile([128, eg, QC], F32)
                g_min = kb_lo - 4 * qi
                g_off0 = 128 * g_min if g_min > 0 else 0
                for kb in range(kb_lo, kb_hi):
                    # write from the group's min offset so the grouped exp
                    # below never reads uninitialized psum
                    off = g_off0
                    qq = off
                    while qq < QC:
                        w = min(256, QC - qq)
                        nc.tensor.matmul(
                            sc_ps[:, kb - kb_lo, qq : qq + w],
                            k_ap(kb * KB, (kb + 1) * KB),
                            q_ap(q0 + qq, q0 + qq + w),
                            start=True,
                            stop=True,
                            perf_mode=DR,
                        )
                        qq += w
                et = expp.tile([128, eg, QC], BF16)
                # cols < 128*m of diagonal block m are never read by
                # AV; a rectangular trim to the group's min offset is
                # safe and cuts ACT work on the causal tail.
                g_min_m = kb_lo - 4 * qi
                g_off = 128 * g_min_m if g_min_m > 0 else 0
                nc.scalar.activation(
                    et[:, 0:gw, g_off:QC],
                    sc_ps[:, 0:gw, g_off:QC],
                    mybir.ActivationFunctionType.Exp,
                    bias=0.0,
                    scale=SCALE / (QK_SCALE * QK_SCALE),
                )
                for kb in range(kb_lo, kb_hi):
                    m = kb - 4 * qi
                    if m >= 0:
                        off = 128 * m
                        nc.vector.tensor_mul(
                            et[:, kb - kb_lo, off : off + 128],
                            et[:, kb - kb_lo, off : off + 128],
                            tri[:, :],
                        )
                exp_ns = gw * (QC - g_off) * ACT_NS + 228.0
                return (h, qi, gi, et, kb_lo, kb_hi, nkb), exp_ns

            def emit_av_group(state):
                h, qi, gi, et, kb_lo, kb_hi, nkb = state
                av_ps = av_tiles[h]
                pe_ns = 0.0
                for kb in range(kb_lo, kb_hi):
                    m = kb - 4 * qi
                    off = 128 * m if m >= 0 else 0
                    nc.tensor.matmul(
                        av_ps[:, off:QC],
                        v_sb[:, kb, h, :],
                        et[:, kb - kb_lo, off:QC],
                        start=(kb == 0),
                        stop=(kb == nkb - 1),
                    )
                    pe_ns += (QC - off) * PE_NS
                if kb_hi == nkb:
                    q0 = qi * QC
                    rbc = smallp.tile([HD, QC], F32)
                    nc.vector.reciprocal(rbc[:, :], av_ps[HD : 2 * HD, :])
                    hc = h // 2         # ctx keeps natural head order
                    hp = 64 * (h % 2)
                    nc.vector.tensor_mul(
                        ctx_sb[hp : hp + HD, hc, q0 : q0 + QC],
                        av_ps[0:HD, :],
                        rbc[:, :],
                    )
                    del av_tiles[h]
                return pe_ns

            _o_tiles = {}

            def emit_ph3_group(qq, eh):
                ps = ps_qkv.tile([128, D // 2], F32, name="ps_op", tag="ps_qkv")
                for cc in range(NCC):
                    nc.tensor.matmul(
                        ps[:, :],
                        ctx_sb[:, cc, qq * 128 : (qq + 1) * 128],
                        wo_sb[:, cc, eh * (D // 2) : (eh + 1) * (D // 2)],
                        start=(cc == 0),
                        stop=(cc == NCC - 1),
                    )
                if eh == 0:
                    _o_tiles[qq] = outp.tile([128, D], F32, name="o_sb")
                o_sb = _o_tiles[qq]
                nc.vector.tensor_copy(o_sb[:, eh * (D // 2) : (eh + 1) * (D // 2)], ps[:, :])
                if eh == 1:
                    nc.sync.dma_start(
                        out=out[qq * 128 : (qq + 1) * 128, :],
                        in_=_o_tiles.pop(qq)[:, :],
                    )
                return NCC * (D // 2) * PE_NS

            def ph1_chunks(tci):
                """Fine-grained filler units for one projection t-chunk."""
                chunks = [lambda tci=tci: emit_xt_load(tci)]
                for is_q in (True, False):
                    for cc in range(NCC):
                        chunks.append(lambda t=tci, q=is_q, c=cc: emit_qk_group(t, q, c))
                for tt in range(T_CHUNK // 128):
                    chunks.append(lambda t=tci, s=tt: emit_v_group(t, s))
                return chunks

            TPQ = QC // T_CHUNK  # t-chunks per attention q-chunk
            QQP = QC // 128      # out-proj 128-row chunks per q-chunk
            # quad-3 heads last: their scores wait on the relocation DMA
            HEAD_ORDER = [0, 1, 2, 4, 5, 6, 3, 7]

            # Estimated arrival times of the serialized scalar-queue input
            # DMAs (all engines' queues share one HWDGE + DMA pipe, so
            # emission order == transfer order). Used to gate filler: work
            # emitted before its inputs exist head-of-line-blocks an
            # in-order queue.
            def _dma_ns(nbytes, elem):
                mult = 2.0 if elem < 512 else 1.0
                return (nbytes / elem) / 16.0 * max(elem * mult / 22.5, 7.0)

            _arr = {}
            _dma_clk = 2300.0
            for _nm, _nb, _el in (
                ("x8_0", D * S // 4, 512), ("wq8", D * C, 512),
                ("wk8", D * C, 512), ("bq", C * 4, 2048),
                ("xt0", D * T_CHUNK * 2, 1024), ("wv", D * C * 2, 1024),
                ("x8_1", D * S // 4, 512), ("bv", 128 * C * 4, 2048),
                ("xt1", D * T_CHUNK * 2, 1024), ("x8_2", D * S // 4, 512),
                ("x8_3", D * S // 4, 512), ("wo", C * D * 2, 2048),
            ):
                _dma_clk += _dma_ns(_nb, _el) + 90.0
                _arr[_nm] = _dma_clk
            _xt_arr = {0: _arr["xt0"], 1: _arr["xt1"]}

            def _ready_qk(tci):
                xq = (tci * T_CHUNK) // (S // 4)
                return max(_arr[f"x8_{xq}"], _arr["wk8"])

            clk = {"pe": 0.0, "act": 0.0}

            def run(chunk):
                clk["pe"] += chunk[1]()

            def pop_filler(filler, limit=None):
                """Emit the first filler chunk whose inputs have arrived;
                returns False if none runnable (or over limit)."""
                for i, ch in enumerate(filler):
                    if ch[0]() <= clk["pe"]:
                        if limit is not None and clk["pe"] + ch[2] > limit + 400.0:
                            continue
                        run(filler.pop(i))
                        return True
                return False

            def emit_block(qi, filler, forced, pre):
                """Phase-2 items for q-chunk qi. AV lags scores/exp by up to
                LAG items; between them, ready filler keeps the PE busy
                while the scalar engine runs exp. `forced` holds (kb_req,
                chunk) pairs emitted before the first AV touching k-block
                kb_req; `pre` maps item index -> chunks emitted before that
                item's scores."""
                nkb = 4 * qi + 4
                n_grp = (nkb + eg_of(qi) - 1) // eg_of(qi)
                pend = []       # up to LAG items awaiting their AV
                # block 0's AVs gate on the value-path DMAs; emit all its
                # scores/exp first so the scalar engine never sits behind
                # a wv-stalled AV in PE counting-semaphore order.
                LAG = 12 if qi == 0 else 3

                def flush_av():
                    state, act_end = pend.pop(0)
                    kb_hi = state[5]
                    while forced and forced[0][0] < kb_hi:
                        clk["pe"] = max(clk["pe"], forced[0][2]())
                        run(forced.pop(0))
                    while clk["pe"] < act_end and pop_filler(filler, limit=act_end):
                        pass
                    clk["pe"] = max(clk["pe"], act_end)
                    clk["pe"] += emit_av_group(state)

                idx = 0
                for h in HEAD_ORDER:
                    for gi in range(n_grp):
                        for ch in pre.pop(idx, ()):
                            clk["pe"] = max(clk["pe"], ch[0]())
                            run(ch)
                        state, exp_ns = emit_sc_group(h, qi, gi)
                        clk["pe"] += sum(sc_widths(qi, gi)) * 0.5 * PE_NS
                        clk["act"] = max(clk["act"], clk["pe"] + 300.0) + exp_ns
                        pend.append((state, clk["act"]))
                        while pend and (len(pend) > LAG or clk["pe"] >= pend[0][1]):
                            flush_av()
                        idx += 1
                # leftover filler first: the pending AVs wait on the block's
                # exp tail anyway, and the next block needs this PE work done
                while filler:
                    if not pop_filler(filler):
                        clk["pe"] = max(clk["pe"] + 1.0, min(c[0]() for c in filler))
                while pend:
                    flush_av()

            for _rep in range(reps):
                # warmup: only the e=0 q/k chunks of t-chunk 0 (all that
                # heads 0-2 of block 0 need); the e=1 chunks run as
                # deadlined filler before head 4's items.
                emit_xt_load(0)
                clk["pe"] = _ready_qk(0)
                for is_q in (True, False):
                    for cc in range(2):
                        clk["pe"] += emit_qk_group(0, is_q, cc)

                def qk_chunk(tci, is_q, cc):
                    def fn():
                        with tc.tile_wait_until(_ready_qk(tci) * 1e-6 * WAIT_SCALE):
                            return emit_qk_group(tci, is_q, cc)
                    return (lambda: _ready_qk(tci), fn, NFP * T_CHUNK * 0.5 * PE_NS)

                def v_chunk(tci, tt):
                    def fn():
                        with tc.tile_wait_until(
                            max(_xt_arr.get(tci, 0.0), _arr["wv"]) * 1e-6 * WAIT_SCALE
                        ):
                            return emit_v_group(tci, tt)
                    return (lambda: max(_xt_arr.get(tci, 1e12), _arr["wv"]), fn,
                            NFC * C * PE_NS)

                def xt_chunk(tci):
                    def fn():
                        emit_xt_load(tci)
                        _xt_arr[tci] = clk["pe"] + 4400.0
                        return 0.0
                    return (lambda: 0.0, fn, 0.0)

                def ph3_chunk(qq, eh):
                    def fn():
                        with tc.tile_wait_until(_arr["wo"] * 1e-6 * WAIT_SCALE):
                            return emit_ph3_group(qq, eh)
                    return (lambda: _arr["wo"], fn, NCC * (D // 2) * PE_NS)

                for qi in range(N_QC):
                    filler = []
                    forced = []
                    pre = {}
                    if qi == 0:
                        pre[3 * ((4 + eg_of(0) - 1) // eg_of(0))] = [qk_chunk(0, q, c) for q in (True, False)
                                  for c in range(2, NCC)]
                        for tt in range(T_CHUNK // 128):
                            vc = v_chunk(0, tt)
                            forced.append((tt, vc[1], vc[0]))
                    if qi + 1 < N_QC:
                        for tci in range(TPQ * (qi + 1), TPQ * (qi + 2)):
                            if tci not in _xt_arr and tci not in _xt_pref:
                                filler.append(xt_chunk(tci))
                            for q in (True, False):
                                for c in range(NCC):
                                    filler.append(qk_chunk(tci, q, c))
                            for tt in range(T_CHUNK // 128):
                                filler.append(v_chunk(tci, tt))
                    if qi > 0:
                        for qq in range((qi - 1) * QQP, qi * QQP):
                            for eh in range(2):
                                filler.append(ph3_chunk(qq, eh))
                    emit_block(qi, filler, forced, pre)
                    for _, fn, _r in forced:
                        fn()
                for qq in range((N_QC - 1) * QQP, N_QC * QQP):
                    for eh in range(2):
                        emit_ph3_group(qq, eh)

    _split_multi_waits(nc)
    return nc


_CACHED = {}


def _build(reps=1):
    if reps not in _CACHED:
        nc = bass.Bass("TRN2", target_bir_lowering=False, debug=False)
        _CACHED[reps] = _emit_kernel(nc, reps)
    return _CACHED[reps]


# q/k column permutation: local column cc*128 + 32*a + r holds head
# 4*(cc//2) + a, contraction dim 32*(cc%2) + r. Applying the same
# permutation to q and k leaves q.k (and so the scores) unchanged.
_QK_PERM = np.empty(C, np.int64)
for _cc in range(C // 128):
    for _a in range(4):
        for _r in range(32):
            _QK_PERM[_cc * 128 + 32 * _a + _r] = (
                (4 * (_cc // 2) + _a) * HD + 32 * (_cc % 2) + _r
            )


def _reference_numpy(x, Wq, bq, Wk, bk, Wv, bv, Wo, bo, attention_mask):
    """Fallback for non-all-ones attention masks (spec fills ones)."""
    scale = HD ** -0.5
    out = np.empty((B, S, D), np.float32)
    causal = np.triu(np.ones((S, S), bool), k=1)
    for b in range(B):
        q = (x[b] @ Wq.T + bq).reshape(S, H, HD).transpose(1, 0, 2)
        k = (x[b] @ Wk.T + bk).reshape(S, H, HD).transpose(1, 0, 2)
        v = (x[b] @ Wv.T + bv).reshape(S, H, HD).transpose(1, 0, 2)
        o = np.empty((H, S, HD), np.float32)
        pad = (attention_mask[b] == 0)[None, :]
        for h in range(H):
            s = (q[h] @ k[h].T) * scale
            s[causal] = -np.inf
            s = np.where(pad, np.float32(-1e9), s)
            s -= s.max(-1, keepdims=True)
            e = np.exp(s)
            p = e / e.sum(-1, keepdims=True)
            o[h] = p @ v[h]
        ctx = o.transpose(1, 0, 2).reshape(S, D)
        out[b] = ctx @ Wo.T + bo
    return out


def kernel(x, Wq, bq, Wk, bk, Wv, bv, Wo, bo, attention_mask):
    x = np.asarray(x, np.float32)
    Wq, bq = np.asarray(Wq, np.float32), np.asarray(bq, np.float32)
    Wk, bk = np.asarray(Wk, np.float32), np.asarray(bk, np.float32)
    Wv, bv = np.asarray(Wv, np.float32), np.asarray(bv, np.float32)
    Wo, bo = np.asarray(Wo, np.float32), np.asarray(bo, np.float32)
    attention_mask = np.asarray(attention_mask)

    if not np.all(attention_mask == 1):
        return _reference_numpy(x, Wq, bq, Wk, bk, Wv, bv, Wo, bo, attention_mask)

    nc = _build()

    xts = [np.ascontiguousarray(x[b].T).astype(ml_dtypes.bfloat16) for b in range(B)]
    xt8s = [np.ascontiguousarray((QK_SCALE * x[b].T)).astype(NP_FP8) for b in range(B)]
    shards = []
    for g in range(2):
        cs = slice(g * C, (g + 1) * C)
        shards.append(
            dict(
                wq8=np.ascontiguousarray((QK_SCALE * Wq[cs, :][_QK_PERM]).T).astype(NP_FP8),
                wk8=np.ascontiguousarray((QK_SCALE * Wk[cs, :][_QK_PERM]).T).astype(NP_FP8),
                wvt=np.ascontiguousarray(Wv[cs, :].T).astype(ml_dtypes.bfloat16),
                bqv=np.ascontiguousarray(QK_SCALE * bq[cs][_QK_PERM]),
                bvb=np.ascontiguousarray(np.broadcast_to(bv[cs], (128, C))),
                wot=np.ascontiguousarray(Wo[:, cs].T).astype(ml_dtypes.bfloat16),
            )
        )
    in_maps = []
    for c in range(N_CORES):
        b, g = c // 2, c % 2
        in_maps.append(dict(xt=xts[b], x8t=xt8s[b], **shards[g]))

    res = run_bass_kernel_spmd(nc, in_maps, core_ids=list(range(N_CORES)))

    out = np.empty((B, S, D), np.float32)
    for b in range(B):
        out[b] = res.results[2 * b]["out"] + res.results[2 * b + 1]["out"] + bo
    return out
